# revision 2
# baseline (speedup 1.0000x reference)
"""Trainium2 Bass kernel for nn_BlockCrossAttn (block-diagonal attention, E=H=1).

Math per (block b, batch n) pair (256-long vectors q', k', v of the block):
    q' = wq*Q + bq ; k' = wk*K + bk
    soft[q,k] = softmax_k(q'[q] * k'[k])
    out[q] = wvo * (sum_k soft[q,k] * V[k]) + (bvo + bo)
where wvo = wo*wv, bvo = wo*bv (the V/out affine folds into the epilogue
because softmax weights sum to 1).  No max-subtraction: |scores| <= ~27
worst case, exp is safe in fp32.

Sharding: 128 blocks of 256 rows; 16 blocks per core across 8 cores
(fully independent, no collectives).

Per-core device pipeline (512 pairs), v2:
  - Scores via bf16 hi/lo splitting: q' = qhi+qlo, k' = khi+klo (each
    split exact to ~2^-16), S = khi*qhi + khi*qlo + klo*qhi (klo*qlo
    ~ 2^-16*|S| dropped).  One 3-row bf16 matmul per (pair, k-half):
    lhsT = [khi; khi; klo][3, 128], rhs = [qhi; qlo; qhi][3, 256]
    -> S^T[k, q] in PSUM.  bf16 weights load ~4x faster than fp32
    (LDWEIGHTS was half the PE queue time in v1).
  - lhsT/rhs operands bulk-staged per block (6 DMAs) from bf16 hi/lo
    prep tiles; no per-16-pair row staging.
  - GROUP=2 pairs per exp: ScalarE exp over [128, 1024] PSUM spans -> E
    (bf16) in SBUF; ps_stage bufs=3 (6 banks) + ps_res bufs=2 (2 banks)
    = all 8 PSUM banks, giving PE two groups of run-ahead so the array
    never waits on the exp.
  - PE reduction matmuls: lhsT = [ones, vhi, vlo] per k-half, rhs = E
    -> PSUM rows (denom, numer_hi, numer_lo) per pair; 4 pairs per
    result bank via tile_position column groups.
  - VectorE flushes banks to SBUF; a DRAM scratch bounce re-lays 32
    pairs into a dense [32, 1536] tile; VectorE adds k-half partials,
    reciprocal_approx_fast + multiply + affine epilogue; one contiguous
    DMA per block to the n-major output.

Weight scalars are baked into the module as immediates (compiled per
weight set, cached) to avoid TensorScalarPtr sync-wait limits.
"""

from contextlib import ExitStack

import numpy as np

import concourse.bacc as bacc
import concourse.bass as bass
import concourse.tile as tile
from concourse import mybir
from concourse.bass_utils import run_bass_kernel_spmd

FP = mybir.dt.float32
AF = mybir.ActivationFunctionType
ALU = mybir.AluOpType
BF16 = mybir.dt.bfloat16

L = 32768          # sequence length
N = 32             # batch
BS = 256           # block size
NB = L // BS       # 128 blocks
NCORES = 8
BPC = NB // NCORES  # 16 blocks per core
LS = BPC * BS       # 4096 rows per core shard

GROUP = 2           # pairs per exp staging group (2 PSUM banks each)
PAIRS = BPC * N     # 512 pairs per core
EDT = BF16          # E dtype for the reductions (rounding cancels in ratio)


def build_kernel_module(sc, reps: int = 1) -> bass.Bass:
    """sc: dict of python-float weight scalars baked as immediates.

    reps > 1 wraps the whole body in a device-side For_i loop — used only
    for wall-clock benchmarking (amplifies device time over dispatch noise).
    """
    nc = bacc.Bacc("TRN2", target_bir_lowering=False, debug=False, num_devices=NCORES)
    # qkt[4n+c, :] = [qT[n, 1024c:1024(c+1)] | kT[n, 1024c:1024(c+1)]]
    qkt = nc.declare_dram_parameter("qkt", [128, 2048], FP, isOutput=False)
    v = nc.declare_dram_parameter("v", [LS, N], FP, isOutput=False)
    out_t = nc.declare_dram_parameter("out_t", [N, LS], FP, isOutput=True)

    with tile.TileContext(nc) as tc:
        with ExitStack() as ctx:
            if reps == 1:
                _emit(ctx, tc, qkt, v, out_t, sc)
            else:
                with tc.For_i(0, reps, 1):
                    _emit(ctx, tc, qkt, v, out_t, sc)
    nc.compile()
    return nc


def _emit(ctx, tc, qkt, v, out_t, sc):
    nc = tc.nc

    rows = ctx.enter_context(tc.tile_pool(name="rows", bufs=1))
    stag = ctx.enter_context(tc.tile_pool(name="stag", bufs=2))
    vpool = ctx.enter_context(tc.tile_pool(name="vpool", bufs=1))
    epool = ctx.enter_context(tc.tile_pool(name="epool", bufs=3))
    dpool = ctx.enter_context(tc.tile_pool(name="dpool", bufs=2))
    ps_stage = ctx.enter_context(tc.tile_pool(name="ps_stage", bufs=3, space="PSUM"))
    ps_res = ctx.enter_context(tc.tile_pool(name="ps_res", bufs=2, space="PSUM"))
    drs = ctx.enter_context(tc.tile_pool(name="drs", bufs=2, space="DRAM"))

    # --- prep ------------------------------------------------------------------
    # qk4 holds affine q' (cols 0:1024) and k' (cols 1024:2048), row 4n+c.
    qk4 = rows.tile([128, 2048], FP, name="qk4", tag="qk4")
    nc.sync.dma_start(out=qk4[:], in_=qkt[:])
    nc.vector.tensor_scalar(
        out=qk4[:, 0:1024], in0=qk4[:, 0:1024],
        scalar1=sc["wq"], scalar2=sc["bq"], op0=ALU.mult, op1=ALU.add,
    )
    nc.vector.tensor_scalar(
        out=qk4[:, 1024:2048], in0=qk4[:, 1024:2048],
        scalar1=sc["wk"], scalar2=sc["bk"], op0=ALU.mult, op1=ALU.add,
    )
    # bf16 hi/lo split of q'/k' (exact to ~2^-16 relative).
    hi = rows.tile([128, 2048], BF16, name="hi", tag="hi")
    hi32 = rows.tile([128, 2048], FP, name="hi32p", tag="hi32p")
    lo = rows.tile([128, 2048], BF16, name="lo", tag="lo")
    nc.vector.tensor_copy(hi[:], qk4[:])
    nc.vector.tensor_copy(hi32[:], hi[:])
    nc.vector.tensor_sub(lo[:], qk4[:], hi32[:])

    def hview(T):
        # [n, c, h(q=0/k=1), w=1024]
        return T[:].rearrange("(n c) (h w) -> n c h w", c=4, h=2)

    # [ones, v] tiles: col 0 = 1.0 (memset once); cols 1..2 = bf16 hi/lo of
    # raw V of the block, [t, n] order.  Two fixed tiles alternate per block.
    vcombs = []
    for name in ("vcA", "vcB"):
        vc = vpool.tile([128, 2, N, 3], EDT, name=name, tag=name)
        nc.vector.memset(vc[:], 1.0)
        vcombs.append(vc)

    def load_vcomb(b):
        # DMA raw V, then split into bf16 hi+lo columns (exact to ~2^-16).
        vc = vcombs[b % 2]
        vch = vpool.tile([128, 2, N], FP, name="vch", tag="vch", bufs=2)
        vhi32 = vpool.tile([128, 2, N], FP, name="vhi32", tag="vhi32", bufs=2)
        nc.sync.dma_start(
            out=vch[:],
            in_=v[b * BS:(b + 1) * BS, :].rearrange("(t p) n -> p t n", p=128),
        )
        vc4 = vc[:]
        nc.vector.tensor_copy(vc4[:, :, :, 1], vch[:])
        nc.vector.tensor_copy(vhi32[:], vc4[:, :, :, 1])
        nc.vector.tensor_sub(vc4[:, :, :, 2], vch[:], vhi32[:])
        return vc

    # --- per-block outer-product operand staging ------------------------------
    # lhsS[r, n, t, j]: r in {0,1} = khi, r=2 = klo of k-half t, pair n.
    # rhsS[r, n, j]:    r in {0,2} = qhi, r=1 = qlo of pair n.
    def load_stage(b):
        cb, cc = b // 4, (b % 4) * 256
        lhsS = stag.tile([3, N, 2, 128], BF16, name="lhsS", tag="lhsS")
        rhsS = stag.tile([3, N, 256], BF16, name="rhsS", tag="rhsS")
        khi = hview(hi)[:, cb, 1, cc:cc + 256].rearrange("n (t j) -> n t j", t=2)
        klo = hview(lo)[:, cb, 1, cc:cc + 256].rearrange("n (t j) -> n t j", t=2)
        qhi = hview(hi)[:, cb, 0, cc:cc + 256]
        qlo = hview(lo)[:, cb, 0, cc:cc + 256]
        nc.sync.dma_start(out=lhsS[0:1], in_=khi)
        nc.sync.dma_start(out=lhsS[1:2], in_=khi)
        nc.sync.dma_start(out=lhsS[2:3], in_=klo)
        nc.sync.dma_start(out=rhsS[0:1], in_=qhi)
        nc.sync.dma_start(out=rhsS[1:2], in_=qlo)
        nc.sync.dma_start(out=rhsS[2:3], in_=qhi)
        return lhsS, rhsS

    # --- main loop --------------------------------------------------------------
    vcur = [None]
    res_state = {"tile": None, "count": 0, "nflush": 0, "rs": None, "first_g": 0}

    def emit_reduces(pend):
        e, members = pend
        for (s, b, n, vc) in members:
            g = b * N + n
            r = res_state["count"]
            if r == 0:
                res_state["tile"] = ps_res.tile([128, 512], FP, name="res", tag="res")
                if res_state["nflush"] == 0:
                    res_state["rs"] = dpool.tile([128, 4096], FP, name="rs", tag="rs")
                    res_state["first_g"] = g
            jj = r
            for t in (0, 1):
                nc.tensor.matmul(
                    res_state["tile"][32 * jj:32 * jj + 3, t * 256:(t + 1) * 256],
                    lhsT=vc[:][:, t, n, :],
                    rhs=e[:][:, s * 512 + t * 256: s * 512 + (t + 1) * 256],
                    start=True, stop=True,
                    tile_position=(0, 32 * jj),
                )
            res_state["count"] += 1
            if res_state["count"] == 4:
                m = res_state["nflush"]
                nc.vector.tensor_copy(
                    res_state["rs"][:, m * 512:(m + 1) * 512], res_state["tile"][:]
                )
                res_state["count"] = 0
                res_state["tile"] = None
                res_state["nflush"] += 1
                if res_state["nflush"] == 8:
                    division_batch()

    def division_batch():
        b0 = res_state["first_g"] // N
        rs = res_state["rs"]
        # rows {32j+r} of rs -> DRAM scratch already in dense layout:
        # scr[4m+j, r*512 + tq] ; then scratch -> dn is a contiguous copy.
        scr = drs.tile([N, 1536], FP, name="scr", tag="scr")
        rsv = rs[:].rearrange("(j p2) (m tq) -> j p2 m tq", j=4, m=8)
        sw = scr[:].rearrange("(m j) (r tq) -> j m r tq", m=8, r=3)
        for r in (0, 1, 2):
            nc.sync.dma_start(out=sw[:, :, r, :], in_=rsv[:, r, :, :])
        # scratch -> dense [32, 1536]: partition 4m+j (= local pair n), free (r,t,q)
        dn = dpool.tile([N, 1536], FP, name="dn", tag="dn")
        nc.sync.dma_start(out=dn[:], in_=scr[:])
        dnv = dn[:].rearrange("p (r t q) -> p r t q", r=3, t=2)
        den = dpool.tile([N, BS], FP, name="den", tag="den")
        num = dpool.tile([N, BS], FP, name="num", tag="num")
        nc.vector.tensor_add(den[:], dnv[:, 0, 0, :], dnv[:, 0, 1, :])
        nc.vector.tensor_add(num[:], dnv[:, 1, 0, :], dnv[:, 1, 1, :])
        nc.vector.tensor_add(num[:], num[:], dnv[:, 2, 0, :])
        nc.vector.tensor_add(num[:], num[:], dnv[:, 2, 1, :])
        nc.vector.reciprocal_approx_fast(out=den[:], in_=den[:])
        ov = dpool.tile([N, BS], FP, name="ov", tag="ov")
        nc.vector.tensor_mul(ov[:], num[:], den[:])
        nc.vector.tensor_scalar(
            out=ov[:], in0=ov[:], scalar1=sc["wvo"], scalar2=sc["bvo"] + sc["bo"],
            op0=ALU.mult, op1=ALU.add,
        )
        nc.sync.dma_start(out=out_t[:, b0 * BS:(b0 + 1) * BS], in_=ov[:])
        res_state["nflush"] = 0
        res_state["rs"] = None

    pending = None
    cur_stage = None
    members = []
    stages = {0: load_stage(0)}
    for g in range(PAIRS):
        b, n = divmod(g, N)
        if n == 0:
            vcur[0] = load_vcomb(b)
            if b + 1 < BPC:
                stages[(b + 1) % 2] = load_stage(b + 1)
        lhsS, rhsS = stages[b % 2]
        s = g % GROUP
        if s == 0:
            cur_stage = ps_stage.tile([128, GROUP * 512], FP, name="st", tag="st")
            members = []
        for t in (0, 1):
            nc.tensor.matmul(
                cur_stage[:, s * 512 + t * 256: s * 512 + (t + 1) * 256],
                lhsT=lhsS[:][:, n, t, :],
                rhs=rhsS[:][:, n, :],
                start=True, stop=True,
                tile_position=(0, 0),
            )
        members.append((s, b, n, vcur[0]))
        if s == GROUP - 1 or g == PAIRS - 1:
            e = epool.tile([128, GROUP * 512], EDT, name="e", tag="e")
            width = len(members) * 512
            nc.scalar.activation(e[:][:, 0:width], cur_stage[:][:, 0:width], AF.Exp)
            if pending is not None:
                emit_reduces(pending)
            pending = (e, members)
    emit_reduces(pending)
    assert res_state["count"] == 0 and res_state["nflush"] == 0, (
        "pair count must be a multiple of 32 (one block per division batch)"
    )


_CACHE: dict = {}


def _get_nc(sc, reps: int = 1) -> bass.Bass:
    key = (tuple(sorted(sc.items())), reps)
    if key not in _CACHE:
        _CACHE[key] = build_kernel_module(sc, reps)
    return _CACHE[key]


def make_in_maps(query, key, value, in_proj_w, in_proj_b, out_proj_w, out_proj_b):
    q = np.ascontiguousarray(np.asarray(query, dtype=np.float32).reshape(L, N))
    k = np.ascontiguousarray(np.asarray(key, dtype=np.float32).reshape(L, N))
    vv = np.ascontiguousarray(np.asarray(value, dtype=np.float32).reshape(L, N))
    wq, wk, wv = [float(x) for x in np.asarray(in_proj_w, dtype=np.float32).reshape(3)]
    bq, bk, bv = [float(x) for x in np.asarray(in_proj_b, dtype=np.float32).reshape(3)]
    wo = float(np.asarray(out_proj_w, dtype=np.float32).reshape(1)[0])
    bo = float(np.asarray(out_proj_b, dtype=np.float32).reshape(1)[0])
    sc = {"wq": wq, "bq": bq, "wk": wk, "bk": bk,
          "wvo": float(np.float32(wo) * np.float32(wv)),
          "bvo": float(np.float32(wo) * np.float32(bv)), "bo": bo}
    in_maps = []
    for c in range(NCORES):
        sl = slice(c * LS, (c + 1) * LS)
        qr = np.ascontiguousarray(q[sl].T).reshape(N, 4, LS // 4)
        kr = np.ascontiguousarray(k[sl].T).reshape(N, 4, LS // 4)
        qkt_np = np.concatenate([qr, kr], axis=2).reshape(128, 2048)
        in_maps.append({
            "qkt": np.ascontiguousarray(qkt_np),
            "v": np.ascontiguousarray(vv[sl]),
        })
    return in_maps, sc


def run(in_maps, sc, **kwargs):
    return run_bass_kernel_spmd(_get_nc(sc), in_maps, list(range(NCORES)), **kwargs)


def assemble(results) -> np.ndarray:
    outs = [np.asarray(results[c]["out_t"], dtype=np.float32).T for c in range(NCORES)]
    return np.ascontiguousarray(np.concatenate(outs, axis=0)).reshape(L, N, 1)


def kernel(query, key, value, in_proj_w, in_proj_b, out_proj_w, out_proj_b):
    in_maps, sc = make_in_maps(
        query, key, value, in_proj_w, in_proj_b, out_proj_w, out_proj_b
    )
    res = run(in_maps, sc)
    return assemble(res.results)


# revision 7
# speedup vs baseline: 1.1458x; 1.1458x over previous
"""Trainium2 Bass kernel for nn_BlockCrossAttn (block-diagonal attention, E=H=1).

Math per (block b, batch n) pair (256-long vectors q', k', v of the block):
    q' = wq*Q + bq ; k' = wk*K + bk
    soft[q,k] = softmax_k(q'[q] * k'[k])
    out[q] = wvo * (sum_k soft[q,k] * V[k]) + (bvo + bo)
where wvo = wo*wv, bvo = wo*bv (the V/out affine folds into the epilogue
because softmax weights sum to 1).  No max-subtraction: |scores| <= ~27
worst case, exp is safe in fp32.

Sharding: 128 blocks of 256 rows; 16 blocks per core across 8 cores
(fully independent, no collectives).

Per-core device pipeline (512 pairs):
  - PE outer products (contraction dim 1) build S^T[k, q] in PSUM,
    3 pairs per 3-bank group, double buffered.
  - ScalarE exp over [128, 1536] PSUM spans -> E in SBUF.
  - PE reduction matmuls: one per pair: lhsT = [ones, vhi0, vlo0, vhi1,
    vlo1] 5-column AP, rhs = both E k-halves [128, 512] -> PSUM [5, 512];
    ones row valid over both halves, v rows valid over their own half.
    start=True/stop=True (no PSUM accumulation); 4 pairs per result bank
    via tile_position column groups (concurrent execution).
  - VectorE flushes banks to SBUF; a DRAM scratch bounce re-lays 32 pairs
    into a dense [32, 1024] tile (one writer); VectorE adds the two ktile
    partials, reciprocal_approx_fast + multiply + affine epilogue;
    one contiguous DMA per block to the n-major output.

Weight scalars are baked into the module as immediates (compiled per
weight set, cached) to avoid TensorScalarPtr sync-wait limits.
"""

from contextlib import ExitStack

import numpy as np

import concourse.bacc as bacc
import concourse.bass as bass
import concourse.tile as tile
from concourse import mybir
from concourse.bass_utils import run_bass_kernel_spmd

FP = mybir.dt.float32
AF = mybir.ActivationFunctionType
ALU = mybir.AluOpType

L = 32768          # sequence length
N = 32             # batch
BS = 256           # block size
NB = L // BS       # 128 blocks
NCORES = 8
BPC = NB // NCORES  # 16 blocks per core
LS = BPC * BS       # 4096 rows per core shard

GROUP = 3           # pairs per exp staging group (3 PSUM banks)
PAIRS = BPC * N     # 512 pairs per core
F32R = mybir.dt.float32r
BF16 = mybir.dt.bfloat16
F32R_OUTER = True   # full-rate relaxed-precision fp32 matmul for scores
EDT = BF16          # E dtype for the reductions (rounding cancels in ratio)


def build_kernel_module(sc, reps: int = 1) -> bass.Bass:
    """sc: dict of python-float weight scalars baked as immediates.

    reps > 1 wraps the whole body in a device-side For_i loop — used only
    for wall-clock benchmarking (amplifies device time over dispatch noise).
    """
    nc = bacc.Bacc("TRN2", target_bir_lowering=False, debug=False, num_devices=NCORES)
    # qkt[4n+c, :] = [qT[n, 1024c:1024(c+1)] | kT[n, 1024c:1024(c+1)]]
    qkt = nc.declare_dram_parameter("qkt", [128, 2048], FP, isOutput=False)
    v = nc.declare_dram_parameter("v", [LS, N], FP, isOutput=False)
    out_t = nc.declare_dram_parameter("out_t", [N, LS], FP, isOutput=True)

    with tile.TileContext(nc) as tc:
        with ExitStack() as ctx:
            if reps == 1:
                _emit(ctx, tc, qkt, v, out_t, sc)
            else:
                with tc.For_i(0, reps, 1):
                    _emit(ctx, tc, qkt, v, out_t, sc)
    nc.compile()
    return nc


def _emit(ctx, tc, qkt, v, out_t, sc):
    nc = tc.nc

    rows = ctx.enter_context(tc.tile_pool(name="rows", bufs=1))
    stage = ctx.enter_context(tc.tile_pool(name="stage", bufs=2))
    vpool = ctx.enter_context(tc.tile_pool(name="vpool", bufs=1))
    epool = ctx.enter_context(tc.tile_pool(name="epool", bufs=3))
    dpool = ctx.enter_context(tc.tile_pool(name="dpool", bufs=2))
    ps_stage = ctx.enter_context(tc.tile_pool(name="ps_stage", bufs=2, space="PSUM"))
    ps_res = ctx.enter_context(tc.tile_pool(name="ps_res", bufs=2, space="PSUM"))
    drs = ctx.enter_context(tc.tile_pool(name="drs", bufs=2, space="DRAM"))

    # --- prep ------------------------------------------------------------------
    QKDT = F32R if F32R_OUTER else FP
    qk4 = rows.tile([128, 2048], QKDT, name="qk4", tag="qk4")
    nc.sync.dma_start(out=qk4[:].bitcast(FP), in_=qkt[:])
    nc.vector.tensor_scalar(
        out=qk4[:, 0:1024], in0=qk4[:, 0:1024].bitcast(FP),
        scalar1=sc["wq"], scalar2=sc["bq"], op0=ALU.mult, op1=ALU.add,
    )
    nc.vector.tensor_scalar(
        out=qk4[:, 1024:2048], in0=qk4[:, 1024:2048].bitcast(FP),
        scalar1=sc["wk"], scalar2=sc["bk"], op0=ALU.mult, op1=ALU.add,
    )

    # Per-pair reduce weights [128, n, 5]: col 0 = 1.0 (memset once);
    # cols 1/3 = bf16-hi of V (k-half 0/1), cols 2/4 = bf16-lo residual.
    # One 5-column lhsT covers both k-halves of a pair in a single matmul.
    vcombs = []
    for name in ("vcA", "vcB"):
        vc = vpool.tile([128, N, 5], EDT, name=name, tag=name)
        nc.vector.memset(vc[:], 1.0)
        vcombs.append(vc)

    def load_vcomb(b):
        # DMA raw V, then split into bf16 hi+lo columns (exact to ~2^-16).
        vc = vcombs[b % 2]
        vch = vpool.tile([128, 2, N], FP, name="vch", tag="vch", bufs=2)
        hi32 = vpool.tile([128, N, 2], FP, name="hi32", tag="hi32", bufs=2)
        nc.sync.dma_start(
            out=vch[:],
            in_=v[b * BS:(b + 1) * BS, :].rearrange("(t p) n -> p t n", p=128),
        )
        vchT = vch[:].rearrange("p t n -> p n t")
        vt = vc[:][:, :, 1:5].rearrange("p n (t x) -> p n t x", x=2)
        nc.vector.tensor_copy(vt[:, :, :, 0], vchT)
        nc.vector.tensor_copy(hi32[:], vt[:, :, :, 0])
        nc.vector.tensor_sub(vt[:, :, :, 1], vchT, hi32[:])
        return vc

    # --- per-half-block q/k row staging (to partition 0) -----------------------
    def stage_rows(b, h):
        # row n (16h <= n < 16h+16): q at [0, (2(n-16h))*256:...],
        #                            k at [0, (2(n-16h)+1)*256:...]
        qks = stage.tile([1, 16 * 2 * BS], QKDT, name="qks", tag="qks")
        qv = qk4[:].rearrange("(n c) (g f) -> n c g f", c=4, g=2)
        cb, cc = b // 4, (b % 4) * BS
        nc.sync.dma_start(out=qks[:], in_=qv[16 * h:16 * (h + 1), cb, :, cc:cc + BS])
        return qks

    # --- main loop --------------------------------------------------------------
    vcur = [None]
    res_state = {"tile": None, "count": 0, "nflush": 0, "rs": None, "first_g": 0}

    def emit_reduces(pend):
        e, members = pend
        for (s, b, n, vc) in members:
            g = b * N + n
            r = res_state["count"]
            if r == 0:
                res_state["tile"] = ps_res.tile([128, 512], FP, name="res", tag="res")
                if res_state["nflush"] == 0:
                    res_state["rs"] = dpool.tile([128, 4096], FP, name="rs", tag="rs")
                    res_state["first_g"] = g
            jj = r
            # One matmul per pair: lhsT = [1s, vhi0, vlo0, vhi1, vlo1],
            # rhs = both E k-halves [128, 512].  Row 0 (ones) is valid over
            # both column halves; v rows are valid only over their own half
            # (other half is garbage, ignored in the division epilogue).
            nc.tensor.matmul(
                res_state["tile"][32 * jj:32 * jj + 5, 0:512],
                lhsT=vc[:][:, n, :],
                rhs=e[:][:, s * 512:(s + 1) * 512],
                start=True, stop=True,
                tile_position=(0, 32 * jj),
            )
            res_state["count"] += 1
            if res_state["count"] == 4:
                m = res_state["nflush"]
                nc.vector.tensor_copy(
                    res_state["rs"][:, m * 512:(m + 1) * 512], res_state["tile"][:]
                )
                res_state["count"] = 0
                res_state["tile"] = None
                res_state["nflush"] += 1
                if res_state["nflush"] == 8:
                    division_batch()

    def division_batch():
        b0 = res_state["first_g"] // N
        rs = res_state["rs"]
        # rows {32j+r} of rs -> DRAM scratch already in dense layout:
        # scr[4m+j, r*512 + tq] ; then scratch -> dn is a contiguous copy.
        scr = drs.tile([N, 2560], FP, name="scr", tag="scr")
        rsv = rs[:].rearrange("(j p2) (m tq) -> j p2 m tq", j=4, m=8)
        sw = scr[:].rearrange("(m j) (r tq) -> j m r tq", m=8, r=5)
        for r in range(5):
            nc.sync.dma_start(out=sw[:, :, r, :], in_=rsv[:, r, :, :])
        # scratch -> dense [32, 2560]: partition 4m+j (= local pair n), free (r,t,q)
        dn = dpool.tile([N, 2560], FP, name="dn", tag="dn")
        nc.sync.dma_start(out=dn[:], in_=scr[:])
        dnv = dn[:].rearrange("p (r t q) -> p r t q", r=5, t=2)
        den = dpool.tile([N, BS], FP, name="den", tag="den")
        num = dpool.tile([N, BS], FP, name="num", tag="num")
        # per pair row r, col-half t: r0 = denom (both halves valid);
        # r1/r2 = numer hi/lo of k-half 0 (left half valid);
        # r3/r4 = numer hi/lo of k-half 1 (right half valid).
        nc.vector.tensor_add(den[:], dnv[:, 0, 0, :], dnv[:, 0, 1, :])
        nc.vector.tensor_add(num[:], dnv[:, 1, 0, :], dnv[:, 3, 1, :])
        nc.vector.tensor_add(num[:], num[:], dnv[:, 2, 0, :])
        nc.vector.tensor_add(num[:], num[:], dnv[:, 4, 1, :])
        nc.vector.reciprocal_approx_fast(out=den[:], in_=den[:])
        ov = dpool.tile([N, BS], FP, name="ov", tag="ov")
        nc.vector.tensor_mul(ov[:], num[:], den[:])
        nc.vector.tensor_scalar(
            out=ov[:], in0=ov[:], scalar1=sc["wvo"], scalar2=sc["bvo"] + sc["bo"],
            op0=ALU.mult, op1=ALU.add,
        )
        nc.sync.dma_start(out=out_t[:, b0 * BS:(b0 + 1) * BS], in_=ov[:])
        res_state["nflush"] = 0
        res_state["rs"] = None

    pending = None
    cur_stage = None
    cur_rows = None
    members = []
    for g in range(PAIRS):
        b, n = divmod(g, N)
        if n == 0:
            vcur[0] = load_vcomb(b)
        if n % 16 == 0:
            cur_rows = stage_rows(b, n // 16)
        qks = cur_rows
        nn = n % 16
        s = g % GROUP
        if s == 0:
            cur_stage = ps_stage.tile([128, GROUP * 512], FP, name="st", tag="st")
            members = []
        for t in (0, 1):
            lhsT = qks[:][0:1, (2 * nn + 1) * BS + t * 128: (2 * nn + 1) * BS + (t + 1) * 128]
            rhs = qks[:][0:1, (2 * nn) * BS: (2 * nn + 1) * BS]
            nc.tensor.matmul(
                cur_stage[:, s * 512 + t * 256: s * 512 + (t + 1) * 256],
                lhsT=lhsT, rhs=rhs,
                start=True, stop=True,
                tile_position=(0, 0),
            )
        members.append((s, b, n, vcur[0]))
        if s == GROUP - 1 or g == PAIRS - 1:
            e = epool.tile([128, GROUP * 512], EDT, name="e", tag="e")
            width = len(members) * 512
            nc.scalar.activation(e[:][:, 0:width], cur_stage[:][:, 0:width], AF.Exp)
            if pending is not None:
                emit_reduces(pending)
            pending = (e, members)
    emit_reduces(pending)
    assert res_state["count"] == 0 and res_state["nflush"] == 0, (
        "pair count must be a multiple of 32 (one block per division batch)"
    )


_CACHE: dict = {}


def _get_nc(sc, reps: int = 1) -> bass.Bass:
    key = (tuple(sorted(sc.items())), reps)
    if key not in _CACHE:
        _CACHE[key] = build_kernel_module(sc, reps)
    return _CACHE[key]


def make_in_maps(query, key, value, in_proj_w, in_proj_b, out_proj_w, out_proj_b):
    q = np.ascontiguousarray(np.asarray(query, dtype=np.float32).reshape(L, N))
    k = np.ascontiguousarray(np.asarray(key, dtype=np.float32).reshape(L, N))
    vv = np.ascontiguousarray(np.asarray(value, dtype=np.float32).reshape(L, N))
    wq, wk, wv = [float(x) for x in np.asarray(in_proj_w, dtype=np.float32).reshape(3)]
    bq, bk, bv = [float(x) for x in np.asarray(in_proj_b, dtype=np.float32).reshape(3)]
    wo = float(np.asarray(out_proj_w, dtype=np.float32).reshape(1)[0])
    bo = float(np.asarray(out_proj_b, dtype=np.float32).reshape(1)[0])
    sc = {"wq": wq, "bq": bq, "wk": wk, "bk": bk,
          "wvo": float(np.float32(wo) * np.float32(wv)),
          "bvo": float(np.float32(wo) * np.float32(bv)), "bo": bo}
    in_maps = []
    for c in range(NCORES):
        sl = slice(c * LS, (c + 1) * LS)
        qr = np.ascontiguousarray(q[sl].T).reshape(N, 4, LS // 4)
        kr = np.ascontiguousarray(k[sl].T).reshape(N, 4, LS // 4)
        qkt_np = np.concatenate([qr, kr], axis=2).reshape(128, 2048)
        in_maps.append({
            "qkt": np.ascontiguousarray(qkt_np),
            "v": np.ascontiguousarray(vv[sl]),
        })
    return in_maps, sc


def run(in_maps, sc, **kwargs):
    return run_bass_kernel_spmd(_get_nc(sc), in_maps, list(range(NCORES)), **kwargs)


def assemble(results) -> np.ndarray:
    outs = [np.asarray(results[c]["out_t"], dtype=np.float32).T for c in range(NCORES)]
    return np.ascontiguousarray(np.concatenate(outs, axis=0)).reshape(L, N, 1)


def kernel(query, key, value, in_proj_w, in_proj_b, out_proj_w, out_proj_b):
    in_maps, sc = make_in_maps(
        query, key, value, in_proj_w, in_proj_b, out_proj_w, out_proj_b
    )
    res = run(in_maps, sc)
    return assemble(res.results)

